# revision 2
# baseline (speedup 1.0000x reference)
"""Trainium2 Bass kernel for causal self-attention with LoRA on q/v.

Reference shapes: hidden_states [4, 2048, 1024], 16 heads x 64 dims,
LoRA rank 8 (scale 2.0) on q and v projections.

Sharding: 8 cores = 4 batches x 2 head-groups. Core c handles batch
c//2 and heads (c%2)*8 .. (c%2)*8+8, i.e. output channels
(c%2)*512 .. +512 of its batch. Each core's output is disjoint, so the
full output is assembled host-side with no device collectives.

Per-core kernel (all matmuls bf16, fp32 accumulation):
  - q^T/k^T projections:  [dh=128-chunk, t] = W_chunk^T.T @ x^T. LoRA and
    both biases are folded into one extra K=128 matmul chunk against a
    staging tile (qA@x^T rows 0-7, vA@x^T rows 8-15, ones row 32); the
    q/v B-matrices and biases live in matching rows of the stage-2
    stationaries, so the q epilogue is a plain PSUM->SBUF copy.
  - v projection in [t, dh] orientation; epilogue scatters v into a
    [s-chunk, head, 65] buffer whose column 64 is constant 1.0.
  - attention per head PAIR: scores^T [s=128 block, t] for heads 2j and
    2j+1 are computed by row-tiled K=64 matmuls at tile positions (0,0)
    and (64,0), which execute concurrently on the PE array (no K=128
    zero padding). exp on ScalarE with scale=1/8 and the additive mask
    as per-partition bias; causal handled by skipping fully-masked
    blocks plus one [128,128] upper-triangular mask multiply per
    diagonal block.
  - PV: out[t-block, 0:64] += expS^T_chunk.T @ [v | 1]; column 64
    accumulates the softmax denominator. DVE reciprocal + scale, DMA out.
"""

import sys

if "/opt/trn_rl_repo" not in sys.path:
    sys.path.insert(0, "/opt/trn_rl_repo")

import numpy as np
import ml_dtypes

BF16 = ml_dtypes.bfloat16

B, T, H, NH, DH = 4, 2048, 1024, 16, 64
N_CORES = 8
HPC = 8          # heads per core
CH = HPC * DH    # 512 output channels per core
LORA_SCALE = 2.0

_cached = {}


def _build_nc():
    import concourse.bass as bass
    import concourse.mybir as mybir
    from concourse import bacc
    from concourse.tile import TileContext

    dt = mybir.dt
    AF = mybir.ActivationFunctionType

    nc = bacc.Bacc()

    xT_d = nc.dram_tensor("xT", [4, 128, 8, 512], dt.bfloat16, kind="ExternalInput")
    wqT_d = nc.dram_tensor("wqT", [128, 8, 512], dt.bfloat16, kind="ExternalInput")
    wkT_d = nc.dram_tensor("wkT", [128, 8, 512], dt.bfloat16, kind="ExternalInput")
    wvT_d = nc.dram_tensor("wvT", [128, 8, 512], dt.bfloat16, kind="ExternalInput")
    bk_d = nc.dram_tensor("bk", [128, 4], dt.float32, kind="ExternalInput")
    loraA_d = nc.dram_tensor("loraA", [128, 8, 16], dt.bfloat16, kind="ExternalInput")
    qBsT_d = nc.dram_tensor("qBsT", [128, 512], dt.bfloat16, kind="ExternalInput")
    vBa_d = nc.dram_tensor("vBa", [128, 512], dt.bfloat16, kind="ExternalInput")
    amask_d = nc.dram_tensor("amask", [128, 16], dt.float32, kind="ExternalInput")
    tri_d = nc.dram_tensor("tri", [128, 128], dt.bfloat16, kind="ExternalInput")
    out_d = nc.dram_tensor("out", [T, CH], dt.float32, kind="ExternalOutput")

    with TileContext(nc) as tc:
        with (
            tc.tile_pool(name="const", bufs=1) as cpool,
            tc.tile_pool(name="big", bufs=1) as bpool,
            tc.tile_pool(name="small", bufs=6) as spool,
            tc.tile_pool(name="psproj", bufs=2, space="PSUM") as ps_proj,
            tc.tile_pool(name="pssc", bufs=1, space="PSUM") as ps_sc,
        ):
            # ---- persistent SBUF tensors -------------------------------
            amask_sb = cpool.tile([128, 16], dt.float32, tag="amask")
            nc.sync.dma_start(amask_sb[:], amask_d[:])
            tri_sb = cpool.tile([128, 128], dt.bfloat16, tag="tri")
            nc.sync.dma_start(tri_sb[:], tri_d[:])
            bk_sb = cpool.tile([128, 4], dt.float32, tag="bk")
            nc.sync.dma_start(bk_sb[:], bk_d[:])
            loraA_sb = cpool.tile([128, 8, 16], dt.bfloat16, tag="loraA")
            nc.sync.dma_start(loraA_sb[:], loraA_d[:])
            qBsT_sb = cpool.tile([128, 512], dt.bfloat16, tag="qBsT")
            nc.sync.dma_start(qBsT_sb[:], qBsT_d[:])
            vBa_sb = cpool.tile([128, 512], dt.bfloat16, tag="vBa")
            nc.sync.dma_start(vBa_sb[:], vBa_d[:])

            x_sb = [[None] * 8 for _ in range(4)]
            def load_x(tb):
                for kc in range(8):
                    xt = bpool.tile(
                        [128, 512], dt.bfloat16, tag=f"x{tb}_{kc}", name=f"x{tb}_{kc}"
                    )
                    nc.sync.dma_start(xt[:], xT_d[tb, :, kc, :])
                    x_sb[tb][kc] = xt
            load_x(0)
            wq_sb = bpool.tile([128, 8, 512], dt.bfloat16, tag="wq")
            nc.sync.dma_start(wq_sb[:], wqT_d[:])
            wk_sb = bpool.tile([128, 8, 512], dt.bfloat16, tag="wk")
            nc.sync.dma_start(wk_sb[:], wkT_d[:])
            for tb in range(1, 4):
                load_x(tb)
            wv_sb = bpool.tile([128, 8, 512], dt.bfloat16, tag="wv")
            nc.sync.dma_start(wv_sb[:], wvT_d[:])

            # LoRA stage-1 staging: rows 0-7 = qA @ x^T, rows 8-15 =
            # vA @ x^T, row 32 = ones (carries bq / bv via the stage-2
            # stationaries), everything else zero.
            lql_t = []
            for tb in range(4):
                a = cpool.tile([128, 512], dt.bfloat16, tag=f"lql{tb}", name=f"lql{tb}")
                nc.gpsimd.memset(a[:], 0.0)
                nc.gpsimd.memset(a[32:33, :], 1.0)
                lql_t.append(a)

            qt = [
                [
                    bpool.tile([128, 512], dt.bfloat16, tag=f"q{j}_{tb}", name=f"qt{j}_{tb}")
                    for tb in range(4)
                ]
                for j in range(4)
            ]
            kt = [
                [
                    bpool.tile([128, 512], dt.bfloat16, tag=f"k{j}_{tb}", name=f"kt{j}_{tb}")
                    for tb in range(4)
                ]
                for j in range(4)
            ]
            v_t = []
            for m in range(16):
                vt = bpool.tile([128, 8, 65], dt.bfloat16, tag=f"v{m}", name=f"v{m}")
                nc.gpsimd.memset(vt[:, :, 64:65], 1.0)
                v_t.append(vt)

            # ---- LoRA stage 1: [qA(0:8); vA(8:16)] @ x^T --------------
            def lora1_piece(tb):
                pl = ps_proj.tile([16, 512], dt.float32, tag="proj", name="pl")
                for kc in range(8):
                    nc.tensor.matmul(
                        pl[:],
                        loraA_sb[:, kc, :],
                        x_sb[tb][kc][:],
                        start=(kc == 0),
                        stop=(kc == 7),
                    )
                nc.vector.tensor_copy(lql_t[tb][0:16, :], pl[0:16, :])

            # ---- q/k projections (transposed): [dh-chunk, t] -----------
            def proj_q_piece(j, tb):
                ms = slice(j * 128, (j + 1) * 128)
                pq = ps_proj.tile([128, 512], dt.float32, tag="proj", name="pq")
                for kc in range(8):
                    nc.tensor.matmul(
                        pq[:],
                        wq_sb[:, kc, ms],
                        x_sb[tb][kc][:],
                        start=(kc == 0),
                        stop=False,
                    )
                nc.tensor.matmul(
                    pq[:], qBsT_sb[:, ms], lql_t[tb][:], start=False, stop=True
                )
                nc.vector.tensor_copy(qt[j][tb][:], pq[:])

            def proj_k_piece(j, tb):
                ms = slice(j * 128, (j + 1) * 128)
                pk = ps_proj.tile([128, 512], dt.float32, tag="proj", name="pk")
                for kc in range(8):
                    nc.tensor.matmul(
                        pk[:],
                        wk_sb[:, kc, ms],
                        x_sb[tb][kc][:],
                        start=(kc == 0),
                        stop=(kc == 7),
                    )
                nc.vector.tensor_scalar_add(
                    kt[j][tb][:], pk[:], bk_sb[:, j : j + 1]
                )

            # ---- v projection: [t-chunk, dh] ---------------------------
            def proj_v(m):
                pv = ps_proj.tile([128, 512], dt.float32, tag="proj", name="pv")
                msl = slice((m % 4) * 128, (m % 4 + 1) * 128)
                for kc in range(8):
                    nc.tensor.matmul(
                        pv[:],
                        x_sb[m // 4][kc][:, msl],
                        wv_sb[:, kc, :],
                        start=(kc == 0),
                        stop=False,
                    )
                nc.tensor.matmul(
                    pv[:],
                    lql_t[m // 4][:, (m % 4) * 128 : (m % 4 + 1) * 128],
                    vBa_sb[:],
                    start=False,
                    stop=True,
                )
                nc.vector.tensor_copy(
                    v_t[m][:, :, 0:64], pv[:].rearrange("p (h d) -> p h d", h=8)
                )

            # ---- paired attention scores: heads 2j / 2j+1 --------------
            # Row-tiled K=64 matmuls at tile positions (0,0) and (64,0)
            # run concurrently on the PE array.
            rot = [0]

            def attn_scores_pair(j, sb, etsA, etsB):
                w = 2048 - sb * 128
                etA = spool.tile(
                    [128, w], dt.bfloat16, tag=f"eA{sb}", name=f"eA{sb}", bufs=1
                )
                etB = spool.tile(
                    [128, w], dt.bfloat16, tag=f"eB{sb}", name=f"eB{sb}", bufs=1
                )
                etsA.append(etA)
                etsB.append(etB)
                ssl = slice((sb % 4) * 128, (sb % 4 + 1) * 128)
                lhsA = kt[j][sb // 4][0:64, ssl]
                lhsB = kt[j][sb // 4][64:128, ssl]
                diag_c = (sb * 128) // 512
                for ht in range(2):
                    c_lo = max(2 * ht, diag_c)
                    c_hi = 2 * ht + 2
                    if c_lo >= c_hi:
                        continue
                    scA = ps_sc.tile(
                        [128, 1024], dt.float32, tag=f"sc{rot[0] % 3}", name="scA"
                    )
                    rot[0] += 1
                    scB = ps_sc.tile(
                        [128, 1024], dt.float32, tag=f"sc{rot[0] % 3}", name="scB"
                    )
                    rot[0] += 1
                    for c in range(c_lo, c_hi):
                        r = sb * 128 - c * 512 if c == diag_c else 0
                        ps0 = (c - 2 * ht) * 512 + r
                        ps1 = (c - 2 * ht + 1) * 512
                        nc.tensor.matmul(
                            scA[:, ps0:ps1],
                            lhsA,
                            qt[j][c][0:64, r:512],
                            start=True,
                            stop=True,
                        )
                        nc.tensor.matmul(
                            scB[:, ps0:ps1],
                            lhsB,
                            qt[j][c][64:128, r:512],
                            start=True,
                            stop=True,
                        )
                    off_in = max(0, sb * 128 - ht * 1024)
                    wv_ = 1024 - off_in
                    off_out = ht * 1024 + off_in - sb * 128
                    nc.scalar.activation(
                        etA[:, off_out : off_out + wv_],
                        scA[:, off_in : 1024],
                        AF.Exp,
                        bias=amask_sb[:, sb : sb + 1],
                        scale=0.125,
                    )
                    nc.scalar.activation(
                        etB[:, off_out : off_out + wv_],
                        scB[:, off_in : 1024],
                        AF.Exp,
                        bias=amask_sb[:, sb : sb + 1],
                        scale=0.125,
                    )
                nc.vector.tensor_mul(etA[:, 0:128], etA[:, 0:128], tri_sb[:])
                nc.vector.tensor_mul(etB[:, 0:128], etB[:, 0:128], tri_sb[:])

            def attn_pv(h, m, exp_tiles):
                po_ps = ps_proj.tile([128, 65], dt.float32, tag="proj", name="po_ps")
                for s2 in range(m + 1):
                    off = (m - s2) * 128
                    nc.tensor.matmul(
                        po_ps[:],
                        exp_tiles[s2][:, off : off + 128],
                        v_t[s2][:, h, :],
                        start=(s2 == 0),
                        stop=(s2 == m),
                    )
                rz = spool.tile([128, 1], dt.float32, tag="rz", name="rz")
                nc.vector.reciprocal(rz[:], po_ps[:, 64:65])
                ot = spool.tile([128, 64], dt.float32, tag="ot", name="ot")
                nc.vector.tensor_scalar_mul(ot[:], po_ps[:, 0:64], rz[:])
                nc.sync.dma_start(
                    out_d[m * 128 : (m + 1) * 128, h * 64 : (h + 1) * 64], ot[:]
                )

            def attn_pair(j, fillers, pops_per_sb):
                fq = list(fillers)
                etsA, etsB = [], []
                for sb in range(16):
                    for _ in range(pops_per_sb):
                        if fq:
                            fq.pop(0)()
                    if sb >= 3:
                        attn_pv(2 * j, sb - 3, etsA)
                        attn_pv(2 * j + 1, sb - 3, etsB)
                    attn_scores_pair(j, sb, etsA, etsB)
                for f in fq:
                    f()
                for m in range(13, 16):
                    attn_pv(2 * j, m, etsA)
                    attn_pv(2 * j + 1, m, etsB)

            # ---- schedule ---------------------------------------------
            for tb in range(4):
                lora1_piece(tb)
                proj_q_piece(0, tb)
            proj_k_piece(0, 0)

            def qp(j, tb):
                return lambda: proj_q_piece(j, tb)

            def kp(j, tb):
                return lambda: proj_k_piece(j, tb)

            fill0 = (
                [kp(0, 1), lambda: proj_v(0), lambda: proj_v(1),
                 kp(0, 2), lambda: proj_v(2), lambda: proj_v(3),
                 kp(0, 3)]
                + [lambda m=m: proj_v(m) for m in range(4, 16)]
                + [qp(1, tb) for tb in range(4)]
                + [kp(1, tb) for tb in range(4)]
            )
            fill1 = [qp(2, tb) for tb in range(4)] + [kp(2, tb) for tb in range(4)]
            fill2 = [qp(3, tb) for tb in range(4)] + [kp(3, tb) for tb in range(4)]

            attn_pair(0, fill0, 2)
            attn_pair(1, fill1, 1)
            attn_pair(2, fill2, 1)
            attn_pair(3, [], 0)

    nc.compile()
    return nc


def _prep_core_inputs(c, x, mask, Wq, bq, Wk, bk, Wv, bv, qA, qB, vA, vB):
    b, half = divmod(c, 2)
    hs = half * CH

    xT = np.ascontiguousarray(x[b].T.astype(BF16))  # [1024, 2048]
    xTd = np.ascontiguousarray(xT.reshape(8, 128, 4, 512).transpose(2, 1, 0, 3))

    def wT(W):
        Ws = W[hs : hs + CH]  # [512, 1024]
        return np.ascontiguousarray(
            Ws.T.astype(BF16).reshape(8, 128, 512).transpose(1, 0, 2)
        )

    bk_a = np.ascontiguousarray(
        bk[hs : hs + CH].reshape(4, 128).T.astype(np.float32)
    )  # [128, 4]

    A = np.zeros((16, H), np.float32)
    A[0:8] = qA
    A[8:16] = vA
    loraA = np.ascontiguousarray(
        A.T.astype(BF16).reshape(8, 128, 16).transpose(1, 0, 2)
    )

    qBsT = np.zeros((128, 512), BF16)
    qBsT[0:8] = (LORA_SCALE * qB[hs : hs + CH].T).astype(BF16)
    qBsT[32] = bq[hs : hs + CH].astype(BF16)
    vBa = np.zeros((128, 512), BF16)
    vBa[8:16] = (LORA_SCALE * vB[hs : hs + CH].T).astype(BF16)
    vBa[32] = bv[hs : hs + CH].astype(BF16)

    amask = np.ascontiguousarray(
        mask[b, 0, 0].reshape(16, 128).T.astype(np.float32)
    )
    tri = np.triu(np.ones((128, 128), BF16))

    return {
        "xT": xTd,
        "wqT": wT(Wq),
        "wkT": wT(Wk),
        "wvT": wT(Wv),
        "bk": bk_a,
        "loraA": loraA,
        "qBsT": qBsT,
        "vBa": vBa,
        "amask": amask,
        "tri": tri,
    }


def _run(inputs, trace=False, trace_kwargs=None):
    from concourse.bass_utils import run_bass_kernel_spmd

    args = {k: np.asarray(v) for k, v in inputs.items()}
    in_maps = [
        _prep_core_inputs(
            c,
            args["hidden_states"],
            args["attention_mask"],
            args["Wq"], args["bq"], args["Wk"], args["bk"], args["Wv"], args["bv"],
            args["qA"], args["qB"], args["vA"], args["vB"],
        )
        for c in range(N_CORES)
    ]

    if "nc" not in _cached:
        _cached["nc"] = _build_nc()
    nc = _cached["nc"]

    res = run_bass_kernel_spmd(
        nc, in_maps, core_ids=list(range(N_CORES)), trace=trace,
        **(trace_kwargs or {}),
    )

    full = np.empty((B, T, H), np.float32)
    for c in range(N_CORES):
        b, half = divmod(c, 2)
        full[b, :, half * CH : (half + 1) * CH] = res.results[c]["out"]
    return full, res


def kernel(**inputs):
    full, _ = _run(inputs, trace=False)
    return full


# revision 5
# speedup vs baseline: 1.0684x; 1.0684x over previous
"""Trainium2 Bass kernel for causal self-attention with LoRA on q/v.

Reference shapes: hidden_states [4, 2048, 1024], 16 heads x 64 dims,
LoRA rank 8 (scale 2.0) on q and v projections.

Sharding: 8 cores = 4 batches x 2 head-groups. Core c handles batch
c//2 and heads (c%2)*8 .. (c%2)*8+8, i.e. output channels
(c%2)*512 .. +512 of its batch. Each core's output is disjoint, so the
full output is assembled host-side with no device collectives.

Per-core kernel (all matmuls bf16, fp32 accumulation):
  - q^T/k^T projections:  [dh=128-chunk, t] = W_chunk^T.T @ x^T. LoRA and
    both biases are folded into one extra K=128 matmul chunk against a
    staging tile (qA@x^T rows 0-7, vA@x^T rows 8-15, ones row 32); the
    q/v B-matrices and biases live in matching rows of the stage-2
    stationaries, so the q epilogue is a plain PSUM->SBUF copy.
  - v projection in [t, dh] orientation; epilogue scatters v into a
    [s-chunk, head, 65] buffer whose column 64 is constant 1.0.
  - attention per head PAIR: scores^T [s=128 block, t] for heads 2j and
    2j+1 are computed by row-tiled K=64 matmuls at tile positions (0,0)
    and (64,0), which execute concurrently on the PE array (no K=128
    zero padding). exp on ScalarE with scale=1/8 and the additive mask
    as per-partition bias; causal handled by skipping fully-masked
    blocks plus one [128,128] upper-triangular mask multiply per
    diagonal block.
  - PV: out[t-block, 0:64] += expS^T_chunk.T @ [v | 1]; column 64
    accumulates the softmax denominator. DVE reciprocal + scale, DMA out.
"""

import sys

if "/opt/trn_rl_repo" not in sys.path:
    sys.path.insert(0, "/opt/trn_rl_repo")

import numpy as np
import ml_dtypes

BF16 = ml_dtypes.bfloat16

B, T, H, NH, DH = 4, 2048, 1024, 16, 64
N_CORES = 8
HPC = 8          # heads per core
CH = HPC * DH    # 512 output channels per core
LORA_SCALE = 2.0

_cached = {}


def _build_nc():
    import concourse.bass as bass
    import concourse.mybir as mybir
    from concourse import bacc
    from concourse.tile import TileContext

    dt = mybir.dt
    AF = mybir.ActivationFunctionType

    nc = bacc.Bacc()

    xT_d = nc.dram_tensor("xT", [4, 128, 8, 512], dt.bfloat16, kind="ExternalInput")
    wqT_d = nc.dram_tensor("wqT", [128, 8, 512], dt.bfloat16, kind="ExternalInput")
    wkT_d = nc.dram_tensor("wkT", [128, 8, 512], dt.bfloat16, kind="ExternalInput")
    wvT_d = nc.dram_tensor("wvT", [128, 8, 512], dt.bfloat16, kind="ExternalInput")
    bk_d = nc.dram_tensor("bk", [128, 4], dt.float32, kind="ExternalInput")
    loraA_d = nc.dram_tensor("loraA", [128, 8, 16], dt.bfloat16, kind="ExternalInput")
    qBsT_d = nc.dram_tensor("qBsT", [128, 512], dt.bfloat16, kind="ExternalInput")
    vBa_d = nc.dram_tensor("vBa", [128, 512], dt.bfloat16, kind="ExternalInput")
    amask_d = nc.dram_tensor("amask", [128, 16], dt.float32, kind="ExternalInput")
    tri_d = nc.dram_tensor("tri", [128, 128], dt.bfloat16, kind="ExternalInput")
    out_d = nc.dram_tensor("out", [T, CH], dt.float32, kind="ExternalOutput")

    with TileContext(nc) as tc:
        with (
            tc.tile_pool(name="const", bufs=1) as cpool,
            tc.tile_pool(name="big", bufs=1) as bpool,
            tc.tile_pool(name="small", bufs=6) as spool,
            tc.tile_pool(name="psproj", bufs=2, space="PSUM") as ps_proj,
            tc.tile_pool(name="pssc", bufs=1, space="PSUM") as ps_sc,
        ):
            # ---- persistent SBUF tensors -------------------------------
            amask_sb = cpool.tile([128, 16], dt.float32, tag="amask")
            nc.sync.dma_start(amask_sb[:], amask_d[:])
            tri_sb = cpool.tile([128, 128], dt.bfloat16, tag="tri")
            nc.sync.dma_start(tri_sb[:], tri_d[:])
            bk_sb = cpool.tile([128, 4], dt.float32, tag="bk")
            nc.sync.dma_start(bk_sb[:], bk_d[:])
            loraA_sb = cpool.tile([128, 8, 16], dt.bfloat16, tag="loraA")
            nc.sync.dma_start(loraA_sb[:], loraA_d[:])
            qBsT_sb = cpool.tile([128, 512], dt.bfloat16, tag="qBsT")
            nc.sync.dma_start(qBsT_sb[:], qBsT_d[:])
            vBa_sb = cpool.tile([128, 512], dt.bfloat16, tag="vBa")
            nc.sync.dma_start(vBa_sb[:], vBa_d[:])

            x_sb = [[None] * 8 for _ in range(4)]
            def load_x(tb):
                for kc in range(8):
                    xt = bpool.tile(
                        [128, 512], dt.bfloat16, tag=f"x{tb}_{kc}", name=f"x{tb}_{kc}"
                    )
                    nc.sync.dma_start(xt[:], xT_d[tb, :, kc, :])
                    x_sb[tb][kc] = xt
            load_x(0)
            wq_sb = bpool.tile([128, 8, 512], dt.bfloat16, tag="wq")
            nc.sync.dma_start(wq_sb[:], wqT_d[:])
            wk_sb = bpool.tile([128, 8, 512], dt.bfloat16, tag="wk")
            nc.sync.dma_start(wk_sb[:], wkT_d[:])
            for tb in range(1, 4):
                load_x(tb)
            wv_sb = bpool.tile([128, 8, 512], dt.bfloat16, tag="wv")
            nc.sync.dma_start(wv_sb[:], wvT_d[:])

            # LoRA stage-1 staging: rows 0-7 = qA @ x^T, rows 8-15 =
            # vA @ x^T, row 32 = ones (carries bq / bv via the stage-2
            # stationaries), everything else zero.
            lql_t = []
            for tb in range(4):
                a = cpool.tile([128, 512], dt.bfloat16, tag=f"lql{tb}", name=f"lql{tb}")
                nc.gpsimd.memset(a[:], 0.0)
                nc.gpsimd.memset(a[32:33, :], 1.0)
                lql_t.append(a)

            qt = [
                [
                    bpool.tile([128, 512], dt.bfloat16, tag=f"q{j}_{tb}", name=f"qt{j}_{tb}")
                    for tb in range(4)
                ]
                for j in range(4)
            ]
            kt = [
                [
                    bpool.tile([128, 512], dt.bfloat16, tag=f"k{j}_{tb}", name=f"kt{j}_{tb}")
                    for tb in range(4)
                ]
                for j in range(4)
            ]
            v_t = []
            for m in range(16):
                vt = bpool.tile([128, 8, 65], dt.bfloat16, tag=f"v{m}", name=f"v{m}")
                nc.gpsimd.memset(vt[:, :, 64:65], 1.0)
                v_t.append(vt)

            # ---- LoRA stage 1: [qA(0:8); vA(8:16)] @ x^T --------------
            def lora1_piece(tb):
                pl = ps_proj.tile([16, 512], dt.float32, tag="proj", name="pl")
                for kc in range(8):
                    nc.tensor.matmul(
                        pl[:],
                        loraA_sb[:, kc, :],
                        x_sb[tb][kc][:],
                        start=(kc == 0),
                        stop=(kc == 7),
                    )
                nc.vector.tensor_copy(lql_t[tb][0:16, :], pl[0:16, :])

            # ---- q/k projections (transposed): [dh-chunk, t] -----------
            def proj_q_piece(j, tb):
                ms = slice(j * 128, (j + 1) * 128)
                pq = ps_proj.tile([128, 512], dt.float32, tag="proj", name="pq")
                for kc in range(8):
                    nc.tensor.matmul(
                        pq[:],
                        wq_sb[:, kc, ms],
                        x_sb[tb][kc][:],
                        start=(kc == 0),
                        stop=False,
                    )
                nc.tensor.matmul(
                    pq[:], qBsT_sb[:, ms], lql_t[tb][:], start=False, stop=True
                )
                nc.vector.tensor_copy(qt[j][tb][:], pq[:])

            def proj_k_piece(j, tb):
                ms = slice(j * 128, (j + 1) * 128)
                pk = ps_proj.tile([128, 512], dt.float32, tag="proj", name="pk")
                for kc in range(8):
                    nc.tensor.matmul(
                        pk[:],
                        wk_sb[:, kc, ms],
                        x_sb[tb][kc][:],
                        start=(kc == 0),
                        stop=(kc == 7),
                    )
                nc.vector.tensor_scalar_add(
                    kt[j][tb][:], pk[:], bk_sb[:, j : j + 1]
                )

            # ---- v projection: [t-chunk, dh] ---------------------------
            def proj_v(m):
                pv = ps_proj.tile([128, 512], dt.float32, tag="proj", name="pv")
                msl = slice((m % 4) * 128, (m % 4 + 1) * 128)
                for kc in range(8):
                    nc.tensor.matmul(
                        pv[:],
                        x_sb[m // 4][kc][:, msl],
                        wv_sb[:, kc, :],
                        start=(kc == 0),
                        stop=False,
                    )
                nc.tensor.matmul(
                    pv[:],
                    lql_t[m // 4][:, (m % 4) * 128 : (m % 4 + 1) * 128],
                    vBa_sb[:],
                    start=False,
                    stop=True,
                )
                nc.vector.tensor_copy(
                    v_t[m][:, :, 0:64], pv[:].rearrange("p (h d) -> p h d", h=8)
                )

            # ---- paired attention scores: heads 2j / 2j+1 --------------
            # Row-tiled K=64 matmuls at tile positions (0,0) and (64,0)
            # run concurrently on the PE array. Head A lands in the first
            # PSUM bank of the [128,2,512] tile, head B in the second, so
            # one exp call covers both heads' chunk.
            rot = [0]

            def attn_scores_pair(j, sb, ets):
                w = 2048 - sb * 128
                et = spool.tile(
                    [128, 2, w], dt.bfloat16, tag=f"e{sb}", name=f"e{sb}", bufs=1
                )
                ets.append(et)
                ssl = slice((sb % 4) * 128, (sb % 4 + 1) * 128)
                lhsA = kt[j][sb // 4][0:64, ssl]
                lhsB = kt[j][sb // 4][64:128, ssl]
                diag_c = (sb * 128) // 512
                for c in range(diag_c, 4):
                    r = sb * 128 - c * 512 if c == diag_c else 0
                    sc = ps_sc.tile(
                        [128, 2, 512], dt.float32, tag=f"sc{rot[0] % 3}", name="sc"
                    )
                    rot[0] += 1
                    nc.tensor.matmul(
                        sc[:, 0, r:512],
                        lhsA,
                        qt[j][c][0:64, r:512],
                        start=True,
                        stop=True,
                    )
                    nc.tensor.matmul(
                        sc[:, 1, r:512],
                        lhsB,
                        qt[j][c][64:128, r:512],
                        start=True,
                        stop=True,
                    )
                    off = c * 512 + r - sb * 128
                    nc.scalar.activation(
                        et[:, :, off : off + 512 - r],
                        sc[:, :, r:512],
                        AF.Exp,
                        bias=amask_sb[:, sb : sb + 1],
                        scale=0.125,
                    )
                nc.vector.tensor_mul(et[:, 0, 0:128], et[:, 0, 0:128], tri_sb[:])
                nc.vector.tensor_mul(et[:, 1, 0:128], et[:, 1, 0:128], tri_sb[:])

            def attn_pv(j, p, m, exp_tiles):
                h = 2 * j + p
                po_ps = ps_proj.tile([128, 65], dt.float32, tag="proj", name="po_ps")
                for s2 in range(m + 1):
                    off = (m - s2) * 128
                    nc.tensor.matmul(
                        po_ps[:],
                        exp_tiles[s2][:, p, off : off + 128],
                        v_t[s2][:, h, :],
                        start=(s2 == 0),
                        stop=(s2 == m),
                    )
                rz = spool.tile([128, 1], dt.float32, tag="rz", name="rz")
                nc.vector.reciprocal(rz[:], po_ps[:, 64:65])
                ot = spool.tile([128, 64], dt.float32, tag="ot", name="ot")
                nc.vector.tensor_scalar_mul(ot[:], po_ps[:, 0:64], rz[:])
                nc.sync.dma_start(
                    out_d[m * 128 : (m + 1) * 128, h * 64 : (h + 1) * 64], ot[:]
                )

            # Keep the PE's HAM activity monitor warm when real work runs
            # out: a throwaway matmul into the proj PSUM slot. Idle-looking
            # stretches re-throttle the PE clock to 4/8 and everything
            # after runs at half rate.
            def dummy_mm():
                dm = ps_proj.tile([128, 512], dt.float32, tag="proj", name="dm")
                nc.tensor.matmul(
                    dm[:], tri_sb[:], x_sb[0][0][:], start=True, stop=True
                )

            def attn_pair(j, fillers, pops_per_sb):
                fq = list(fillers)
                ets = []
                for sb in range(16):
                    for _ in range(pops_per_sb):
                        if fq:
                            fq.pop(0)()
                        else:
                            dummy_mm()
                    if sb >= 3:
                        attn_pv(j, 0, sb - 3, ets)
                        attn_pv(j, 1, sb - 3, ets)
                    attn_scores_pair(j, sb, ets)
                for f in fq:
                    f()
                dummy_mm()
                dummy_mm()
                for m in range(13, 16):
                    attn_pv(j, 0, m, ets)
                    attn_pv(j, 1, m, ets)

            # ---- schedule ---------------------------------------------
            for tb in range(4):
                lora1_piece(tb)
                proj_q_piece(0, tb)
            proj_k_piece(0, 0)

            def qp(j, tb):
                return lambda: proj_q_piece(j, tb)

            def kp(j, tb):
                return lambda: proj_k_piece(j, tb)

            fill0 = (
                [kp(0, 1), lambda: proj_v(0), lambda: proj_v(1),
                 kp(0, 2), lambda: proj_v(2), lambda: proj_v(3),
                 kp(0, 3)]
                + [lambda m=m: proj_v(m) for m in range(4, 16)]
                + [qp(1, tb) for tb in range(4)]
                + [kp(1, tb) for tb in range(4)]
            )
            fill1 = [qp(2, tb) for tb in range(4)] + [kp(2, tb) for tb in range(4)]
            fill2 = [qp(3, tb) for tb in range(4)] + [kp(3, tb) for tb in range(4)]

            attn_pair(0, fill0, 2)
            attn_pair(1, fill1, 2)
            attn_pair(2, fill2, 2)
            attn_pair(3, [], 4)

    nc.compile()
    return nc


def _prep_core_inputs(c, x, mask, Wq, bq, Wk, bk, Wv, bv, qA, qB, vA, vB):
    b, half = divmod(c, 2)
    hs = half * CH

    xT = np.ascontiguousarray(x[b].T.astype(BF16))  # [1024, 2048]
    xTd = np.ascontiguousarray(xT.reshape(8, 128, 4, 512).transpose(2, 1, 0, 3))

    def wT(W):
        Ws = W[hs : hs + CH]  # [512, 1024]
        return np.ascontiguousarray(
            Ws.T.astype(BF16).reshape(8, 128, 512).transpose(1, 0, 2)
        )

    bk_a = np.ascontiguousarray(
        bk[hs : hs + CH].reshape(4, 128).T.astype(np.float32)
    )  # [128, 4]

    A = np.zeros((16, H), np.float32)
    A[0:8] = qA
    A[8:16] = vA
    loraA = np.ascontiguousarray(
        A.T.astype(BF16).reshape(8, 128, 16).transpose(1, 0, 2)
    )

    qBsT = np.zeros((128, 512), BF16)
    qBsT[0:8] = (LORA_SCALE * qB[hs : hs + CH].T).astype(BF16)
    qBsT[32] = bq[hs : hs + CH].astype(BF16)
    vBa = np.zeros((128, 512), BF16)
    vBa[8:16] = (LORA_SCALE * vB[hs : hs + CH].T).astype(BF16)
    vBa[32] = bv[hs : hs + CH].astype(BF16)

    amask = np.ascontiguousarray(
        mask[b, 0, 0].reshape(16, 128).T.astype(np.float32)
    )
    tri = np.triu(np.ones((128, 128), BF16))

    return {
        "xT": xTd,
        "wqT": wT(Wq),
        "wkT": wT(Wk),
        "wvT": wT(Wv),
        "bk": bk_a,
        "loraA": loraA,
        "qBsT": qBsT,
        "vBa": vBa,
        "amask": amask,
        "tri": tri,
    }


def _run(inputs, trace=False, trace_kwargs=None):
    from concourse.bass_utils import run_bass_kernel_spmd

    args = {k: np.asarray(v) for k, v in inputs.items()}
    in_maps = [
        _prep_core_inputs(
            c,
            args["hidden_states"],
            args["attention_mask"],
            args["Wq"], args["bq"], args["Wk"], args["bk"], args["Wv"], args["bv"],
            args["qA"], args["qB"], args["vA"], args["vB"],
        )
        for c in range(N_CORES)
    ]

    if "nc" not in _cached:
        _cached["nc"] = _build_nc()
    nc = _cached["nc"]

    res = run_bass_kernel_spmd(
        nc, in_maps, core_ids=list(range(N_CORES)), trace=trace,
        **(trace_kwargs or {}),
    )

    full = np.empty((B, T, H), np.float32)
    for c in range(N_CORES):
        b, half = divmod(c, 2)
        full[b, :, half * CH : (half + 1) * CH] = res.results[c]["out"]
    return full, res


def kernel(**inputs):
    full, _ = _run(inputs, trace=False)
    return full
